# revision 16
# baseline (speedup 1.0000x reference)
"""Causal self-attention (B=2, T=2048, D=1024, H=16) on 8 Trainium2 NeuronCores.

Sharding: data-parallel on batch (2-way) x tensor-parallel on heads (4-way):
each core owns one batch's activations and 4 heads (256 channels) of the
QKV / output-projection weights.  Host pre-transposes x and the weight shards
so the kernel needs no on-chip transposes:
  qT/kT = W[heads] @ x.T        (channels on partitions, T on free axis)
  v     = x @ Wv[heads].T       (T on partitions) + ones column (softmax sum)
  expST[j,t] = exp(0.125 * k_h q_h^T)   (kv-position on partitions)
  yT_aug = v_aug.T @ expST      (row 64 = softmax denominator)
  yT     = yT_aug[:64] * recip(denom) broadcast across partitions
  partial out = yT.T @ WpT[heads]  -> [T, D] partial per core, summed on host.
Causality: fully-masked 128-col j-blocks are skipped, partially-masked columns
sliced away, and one 128x128 triangular mask multiplies the diagonal block.
Softmax skips max-subtraction (scores are O(1) by construction).
The two heads of a pair sit at partition bases 0/64 so their K=64 score
matmuls run concurrently in separate PE row-groups; QKV/projection matmul
groups are interleaved into the attention loop as PE filler work so the
ACT-bound exp stream doesn't leave the TensorEngine idle (HAM re-throttle).
"""
import sys, types

for _p in ("/opt/trn_rl_repo",):
    if _p not in sys.path:
        sys.path.append(_p)


def _install_ntff_hook():
    """Register the axon NTFF profile hook that container boot skips when
    antenv.axon_hooks is absent (needed only for profiled runs)."""
    if "antenv.axon_hooks" in sys.modules:
        return
    mod = types.ModuleType("antenv.axon_hooks")
    _h = [None]
    mod.set_axon_ntff_profile_hook = lambda h: _h.__setitem__(0, h)
    mod.get_axon_ntff_profile_hook = lambda: _h[0]
    sys.modules["antenv.axon_hooks"] = mod
    try:
        import antenv
        antenv.axon_hooks = mod
    except Exception:
        pass
    try:
        from trn_agent_boot.trn_boot import _ntff_profile_via_ctypes
        mod.set_axon_ntff_profile_hook(
            _ntff_profile_via_ctypes("/opt/axon/libaxon_pjrt.so"))
    except Exception:
        pass


_install_ntff_hook()

import numpy as np
import ml_dtypes

import concourse.bass as bass
import concourse.tile as tile
from concourse import bacc, mybir, bass_utils

B, T, D, H = 2, 2048, 1024, 16
HD = 64
NHL = 4            # heads per core
C = NHL * HD       # 256 channels per core
DP = 1024          # contraction dim (biases added separately)
KC = DP // 128     # 8
P = 128
BF = mybir.dt.bfloat16
F32 = mybir.dt.float32
bf16 = ml_dtypes.bfloat16

N_CORES = 8


def build_graph():
    nc = bacc.Bacc("TRN2", target_bir_lowering=False, debug=False,
                   num_devices=N_CORES)
    xt_d = nc.dram_tensor("xt", [DP, T], BF, kind="ExternalInput").ap()
    wq_d = nc.dram_tensor("wq", [DP, C], BF, kind="ExternalInput").ap()
    wk_d = nc.dram_tensor("wk", [DP, C], BF, kind="ExternalInput").ap()
    wv_d = nc.dram_tensor("wv", [DP, C], BF, kind="ExternalInput").ap()
    wp_d = nc.dram_tensor("wp", [C, D], BF, kind="ExternalInput").ap()
    tm_d = nc.dram_tensor("tm", [P, P], BF, kind="ExternalInput").ap()
    bqk_d = nc.dram_tensor("bqk", [P, 4], F32, kind="ExternalInput").ap()
    bv_d = nc.dram_tensor("bv", [1, C], F32, kind="ExternalInput").ap()
    out_d = nc.dram_tensor("out", [T, D], BF, kind="ExternalOutput").ap()

    Exp = mybir.ActivationFunctionType.Exp
    Ln = mybir.ActivationFunctionType.Ln

    with tile.TileContext(nc) as tc:
        with tc.tile_pool(name="sing", bufs=1) as sing, \
             tc.tile_pool(name="fill", bufs=1, space="PSUM") as fillps, \
             tc.tile_pool(name="stps", bufs=2, space="PSUM") as stps, \
             tc.tile_pool(name="ytps", bufs=2, space="PSUM") as ytps, \
             tc.tile_pool(name="esb", bufs=3) as esb, \
             tc.tile_pool(name="nrm", bufs=4) as nrm, \
             tc.tile_pool(name="osb", bufs=3) as osb:
            xt_sb = sing.tile([P, KC, T], BF)
            wq_sb = sing.tile([P, KC, C], BF)
            wk_sb = sing.tile([P, KC, C], BF)
            wv_sb = sing.tile([P, KC, C], BF)
            wp_sb = sing.tile([P, 2, D], BF)
            tm_sb = sing.tile([P, P], BF)
            qt_sb = sing.tile([P, 2, T], BF)
            kt_sb = sing.tile([P, 2, T], BF)
            v_sb = sing.tile([P, 16, NHL, HD + 1], BF)
            yt_sb = sing.tile([P, 2, T], BF)
            bias_sb = sing.tile([P, 2, 2], F32)
            bv_row = sing.tile([1, C], F32)
            bvb_sb = sing.tile([P, C], F32)

            xt_r = xt_d.rearrange("(kc p) t -> kc p t", p=P)
            wq_r = wq_d.rearrange("(kc p) c -> p kc c", p=P)
            nc.sync.dma_start(wq_sb[:, :, 0:P], wq_r[:, :, 0:P])
            nc.sync.dma_start(wq_sb[:, :, P:C], wq_r[:, :, P:C])
            for kc in range(KC):
                nc.sync.dma_start(xt_sb[:, kc, :], xt_r[kc])
            nc.sync.dma_start(wk_sb[:], wk_d.rearrange("(kc p) c -> p kc c", p=P))
            nc.sync.dma_start(wv_sb[:], wv_d.rearrange("(kc p) c -> p kc c", p=P))
            nc.sync.dma_start(tm_sb[:], tm_d)
            nc.sync.dma_start(wp_sb[:], wp_d.rearrange("(cc p) o -> p cc o", p=P))
            nc.vector.memset(v_sb[:, :, :, HD:HD + 1], 1.0)
            nc.sync.dma_start(bias_sb[:], bqk_d.rearrange("p (cc r) -> p cc r", cc=2))
            nc.sync.dma_start(bv_row[:], bv_d)
            nc.gpsimd.partition_broadcast(bvb_sb[:], bv_row[:], channels=P)

            # ---- filler groups: QKV projections + output projection --------
            # Emitted into the attention loop to keep the PE fed while the
            # ACT-bound exp stream runs.  Each closure is one PSUM group.
            gidx = [0]

            Ident = mybir.ActivationFunctionType.Identity

            def qk_group(wsb, dst, cc, tb, ridx, use_act=False):
                def emit():
                    g = gidx[0]; gidx[0] += 1
                    ps = fillps.tile([P, 512], F32, tag=f"fl{g % 2}",
                                     name=f"qk_{g}")
                    for kc in range(KC):
                        nc.tensor.matmul(
                            ps[:],
                            wsb[:, kc, cc * P:(cc + 1) * P],
                            xt_sb[:, kc, tb * 512:(tb + 1) * 512],
                            start=(kc == 0), stop=(kc == KC - 1))
                    if use_act:
                        nc.scalar.activation(
                            dst[:, cc, tb * 512:(tb + 1) * 512], ps[:],
                            Ident, bias=bias_sb[:, cc, ridx:ridx + 1])
                    else:
                        nc.vector.tensor_scalar_add(
                            dst[:, cc, tb * 512:(tb + 1) * 512], ps[:],
                            bias_sb[:, cc, ridx:ridx + 1])
                return emit

            def v_group(ji):
                def emit():
                    g = gidx[0]; gidx[0] += 1
                    ps = fillps.tile([P, C], F32, tag=f"fl{g % 2}",
                                     name=f"v_{g}")
                    for kc in range(KC):
                        nc.tensor.matmul(
                            ps[:],
                            xt_sb[:, kc, ji * P:(ji + 1) * P],
                            wv_sb[:, kc, :],
                            start=(kc == 0), stop=(kc == KC - 1))
                    nc.vector.tensor_add(
                        v_sb[:, ji, :, 0:HD],
                        ps[:].rearrange("p (h x) -> p h x", h=NHL),
                        bvb_sb[:].rearrange("p (h x) -> p h x", h=NHL))
                return emit

            def proj_group(t2, ob):
                def emit():
                    g = gidx[0]; gidx[0] += 1
                    pp = fillps.tile([P, 512], F32, tag=f"fl{g % 2}",
                                     name=f"pr_{g}")
                    for cc in range(2):
                        nc.tensor.matmul(
                            pp[:],
                            yt_sb[:, cc, t2 * P:(t2 + 1) * P],
                            wp_sb[:, cc, ob * 512:(ob + 1) * 512],
                            start=(cc == 0), stop=(cc == 1))
                    ot = osb.tile([P, 512], BF, tag="ot", name=f"ot_{g}")
                    nc.vector.tensor_copy(ot[:], pp[:])
                    nc.sync.dma_start(
                        out_d[t2 * P:(t2 + 1) * P, ob * 512:(ob + 1) * 512],
                        ot[:])
                return emit

            def stage_groups(tb, prelude=False):
                # cc=0 (the first head pair) first so attention can start as
                # soon as its q/k copies land; prelude q-copies go to the
                # Scalar engine so they run in parallel with the k-copies
                # on the Vector engine.
                gs = [qk_group(wq_sb, qt_sb, 0, tb, 0, use_act=prelude),
                      qk_group(wk_sb, kt_sb, 0, tb, 1),
                      qk_group(wq_sb, qt_sb, 1, tb, 0, use_act=prelude),
                      qk_group(wk_sb, kt_sb, 1, tb, 1)]
                for ji in range(4 * tb, 4 * tb + 4):
                    gs.append(v_group(ji))
                return gs

            # prelude: everything attention(tb=0) needs
            for g in stage_groups(0, prelude=True):
                g()

            # ---- attention with interleaved fillers ------------------------
            for tb in range(4):
                tsl = slice(tb * 512, (tb + 1) * 512)
                fillers = []
                if tb < 3:
                    fillers += stage_groups(tb + 1)
                if tb >= 1:
                    for t2 in range(4 * (tb - 1), 4 * tb):
                        for ob in range(2):
                            fillers.append(proj_group(t2, ob))
                njc = 4 * tb + 4
                total_iters = 2 * njc
                it = 0
                done = 0
                for hp in (0, 2):
                    pair = (hp, hp + 1)
                    yps = {h: ytps.tile([P, 512], F32, tag="yt",
                                        name=f"yt_{tb}_{h}")
                           for h in pair}

                    def mm1_pair(ji, _tb=tb, _hp=hp, _pair=pair):
                        off = max(0, (ji - 4 * _tb) * P)
                        stp = stps.tile([P, 1024], F32, tag="st",
                                        name=f"st_{_tb}_{_hp}_{ji}")
                        for h in _pair:
                            bse = 64 * (h % 2)
                            cc = h // 2
                            nc.tensor.matmul(
                                stp[:, 512 * (h - _hp) + off:512 * (h - _hp) + 512],
                                kt_sb[bse:bse + 64, cc, ji * P:(ji + 1) * P],
                                qt_sb[bse:bse + 64, cc,
                                      _tb * 512 + off:(_tb + 1) * 512],
                                start=True, stop=True)
                        return stp, off

                    nxt = mm1_pair(0)
                    for ji in range(njc):
                        stp, off = nxt
                        if ji + 1 < njc:
                            nxt = mm1_pair(ji + 1)
                        et = esb.tile([P, 1024], BF, tag="et",
                                      name=f"et_{tb}_{hp}_{ji}")
                        stp3 = stp[:].rearrange("p (g c) -> p g c", g=2)
                        et3 = et[:].rearrange("p (g c) -> p g c", g=2)
                        nc.scalar.activation(et3[:, :, off:], stp3[:, :, off:],
                                             Exp, scale=0.125)
                        if ji >= 4 * tb:
                            nc.vector.tensor_mul(
                                et3[:, :, off:off + P],
                                et3[:, :, off:off + P],
                                tm_sb[:, None, :].to_broadcast([P, 2, P]))
                        for h in pair:
                            nc.tensor.matmul(
                                yps[h][:HD + 1, off:],
                                v_sb[:, ji, h, :],
                                et[:, 512 * (h - hp) + off:512 * (h - hp) + 512],
                                start=(ji == 0), stop=True,
                                skip_group_check=True)
                        if tb == 3 and hp == 2 and ji >= njc - 4:
                            # the j-chunks beyond ji no longer touch columns
                            # < 128*(ji-11), so that quarter of yT_aug is
                            # final: normalize it and chase with its output
                            # projection to keep the kernel tail short
                            qtr = ji - (njc - 4)
                            qs = slice(qtr * P, (qtr + 1) * P)
                            for h in pair:
                                bse = 64 * (h % 2)
                                rec = nrm.tile([1, P], F32, tag="recq",
                                               name=f"recq_{qtr}_{h}")
                                nc.vector.reciprocal(rec[:],
                                                     yps[h][HD:HD + 1, qs])
                                bc = nrm.tile([HD, P], F32, tag="bcq",
                                              name=f"bcq_{qtr}_{h}")
                                nc.gpsimd.partition_broadcast(bc[:], rec[:],
                                                              channels=HD)
                                nc.vector.tensor_mul(
                                    yt_sb[bse:bse + 64, 1,
                                          1536 + qtr * P:1536 + (qtr + 1) * P],
                                    yps[h][0:HD, qs], bc[:])
                            for ob in range(2):
                                proj_group(12 + qtr, ob)()
                        it += 1
                        while done * total_iters < len(fillers) * it:
                            fillers[done]()
                            done += 1
                    if tb == 3 and hp == 2:
                        continue  # fast tail path below
                    # normalization: get yT out of PSUM fast, then divide by
                    # the softmax denominator (row 64) broadcast across rows.
                    # The denominator row is DMA-reshaped to [128, 4] so the
                    # DVE reciprocal runs at free-size 4 instead of 512.
                    for h in pair:
                        bse = 64 * (h % 2)
                        cc = h // 2
                        ya = nrm.tile([HD + 1, 512], F32, tag="ya",
                                      name=f"ya_{tb}_{h}")
                        nc.vector.tensor_copy(ya[:], yps[h][:HD + 1, :])
                        rr = nrm.tile([P, 4], F32, tag="rr",
                                      name=f"rr_{tb}_{h}")
                        nc.sync.dma_start(rr[:], ya[HD:HD + 1, :])
                        rc = nrm.tile([P, 4], F32, tag="rc",
                                      name=f"rc_{tb}_{h}")
                        nc.vector.reciprocal(rc[:], rr[:])
                        rec = nrm.tile([1, 512], F32, tag="rec",
                                      name=f"rec_{tb}_{h}")
                        nc.sync.dma_start(rec[:], rc[:])
                        bc = nrm.tile([HD, 512], F32, tag="bc",
                                      name=f"bc_{tb}_{h}")
                        nc.gpsimd.partition_broadcast(bc[:], rec[:], channels=HD)
                        nc.vector.tensor_mul(yt_sb[bse:bse + 64, cc, tsl],
                                             ya[0:HD, :], bc[:])
                while done < len(fillers):
                    fillers[done]()
                    done += 1



    nc.compile()
    return nc


_NC = None


def _get_nc():
    global _NC
    if _NC is None:
        _NC = build_graph()
    return _NC


def make_in_maps(x, Wq, bq, Wk, bk, Wv, bv, Wp, bp):
    x = np.asarray(x, np.float32)
    tm = np.triu(np.ones((P, P), np.float32)).astype(bf16)  # keep where p <= f
    in_maps = []
    for core in range(N_CORES):
        b = core // 4
        hg = core % 4
        rs = slice(hg * C, (hg + 1) * C)
        xt = np.ascontiguousarray(x[b].T).astype(bf16)
        m = {"xt": xt, "tm": tm}
        for name, W in (("wq", Wq), ("wk", Wk), ("wv", Wv)):
            m[name] = np.ascontiguousarray(
                np.asarray(W, np.float32)[rs].T).astype(bf16)
        m["wp"] = np.ascontiguousarray(
            np.asarray(Wp, np.float32)[:, rs].T).astype(bf16)
        bqs = np.asarray(bq, np.float32)[rs].reshape(2, P)
        bks = np.asarray(bk, np.float32)[rs].reshape(2, P)
        m["bqk"] = np.stack([bqs[0], bks[0], bqs[1], bks[1]], axis=1)
        m["bv"] = np.asarray(bv, np.float32)[rs].reshape(1, C)
        in_maps.append(m)
    return in_maps


def kernel(x, Wq, bq, Wk, bk, Wv, bv, Wp, bp, _trace=False):
    nc = _get_nc()
    in_maps = make_in_maps(x, Wq, bq, Wk, bk, Wv, bv, Wp, bp)
    res = bass_utils.run_bass_kernel_spmd(
        nc, in_maps, core_ids=list(range(N_CORES)), trace=_trace)
    kernel.last_exec_time_ns = res.exec_time_ns
    bp = np.asarray(bp, np.float32)
    out = np.empty((B, T, D), np.float32)
    for b in range(B):
        acc = np.zeros((T, D), np.float32)
        for hg in range(4):
            acc += res.results[4 * b + hg]["out"].astype(np.float32)
        out[b] = acc + bp
    return out


# revision 17
# speedup vs baseline: 1.0576x; 1.0576x over previous
"""Causal self-attention (B=2, T=2048, D=1024, H=16) on 8 Trainium2 NeuronCores.

Sharding: data-parallel on batch (2-way) x tensor-parallel on heads (4-way):
each core owns one batch's activations and 4 heads (256 channels) of the
QKV / output-projection weights.  Host pre-transposes x and the weight shards
so the kernel needs no on-chip transposes:
  qT/kT = W[heads] @ x.T        (channels on partitions, T on free axis)
  v     = x @ Wv[heads].T       (T on partitions) + ones column (softmax sum)
  expST[j,t] = exp(0.125 * k_h q_h^T)   (kv-position on partitions)
  yT_aug = v_aug.T @ expST      (row 64 = softmax denominator)
  yT     = yT_aug[:64] * recip(denom) broadcast across partitions
  partial out = yT.T @ WpT[heads]  -> [T, D] partial per core, summed on host.
Causality: fully-masked 128-col j-blocks are skipped, partially-masked columns
sliced away, and one 128x128 triangular mask multiplies the diagonal block.
Softmax skips max-subtraction (scores are O(1) by construction).
The two heads of a pair sit at partition bases 0/64 so their K=64 score
matmuls run concurrently in separate PE row-groups; QKV/projection matmul
groups are interleaved into the attention loop as PE filler work so the
ACT-bound exp stream doesn't leave the TensorEngine idle (HAM re-throttle).
"""
import sys, types

for _p in ("/opt/trn_rl_repo",):
    if _p not in sys.path:
        sys.path.append(_p)


def _install_ntff_hook():
    """Register the axon NTFF profile hook that container boot skips when
    antenv.axon_hooks is absent (needed only for profiled runs)."""
    if "antenv.axon_hooks" in sys.modules:
        return
    mod = types.ModuleType("antenv.axon_hooks")
    _h = [None]
    mod.set_axon_ntff_profile_hook = lambda h: _h.__setitem__(0, h)
    mod.get_axon_ntff_profile_hook = lambda: _h[0]
    sys.modules["antenv.axon_hooks"] = mod
    try:
        import antenv
        antenv.axon_hooks = mod
    except Exception:
        pass
    try:
        from trn_agent_boot.trn_boot import _ntff_profile_via_ctypes
        mod.set_axon_ntff_profile_hook(
            _ntff_profile_via_ctypes("/opt/axon/libaxon_pjrt.so"))
    except Exception:
        pass


_install_ntff_hook()

import numpy as np
import ml_dtypes

import concourse.bass as bass
import concourse.tile as tile
from concourse import bacc, mybir, bass_utils

B, T, D, H = 2, 2048, 1024, 16
HD = 64
NHL = 4            # heads per core
C = NHL * HD       # 256 channels per core
DP = 1024          # contraction dim (biases added separately)
KC = DP // 128     # 8
P = 128
BF = mybir.dt.bfloat16
F32 = mybir.dt.float32
bf16 = ml_dtypes.bfloat16

N_CORES = 8


def build_graph():
    nc = bacc.Bacc("TRN2", target_bir_lowering=False, debug=False,
                   num_devices=N_CORES)
    xt_d = nc.dram_tensor("xt", [DP, T], BF, kind="ExternalInput").ap()
    wq_d = nc.dram_tensor("wq", [DP, C], BF, kind="ExternalInput").ap()
    wk_d = nc.dram_tensor("wk", [DP, C], BF, kind="ExternalInput").ap()
    wv_d = nc.dram_tensor("wv", [DP, C], BF, kind="ExternalInput").ap()
    wp_d = nc.dram_tensor("wp", [C, D], BF, kind="ExternalInput").ap()
    tm_d = nc.dram_tensor("tm", [P, P], BF, kind="ExternalInput").ap()
    bqk_d = nc.dram_tensor("bqk", [P, 4], F32, kind="ExternalInput").ap()
    bv_d = nc.dram_tensor("bv", [1, C], F32, kind="ExternalInput").ap()
    out_d = nc.dram_tensor("out", [T, D], BF, kind="ExternalOutput").ap()

    Exp = mybir.ActivationFunctionType.Exp
    Ln = mybir.ActivationFunctionType.Ln

    with tile.TileContext(nc) as tc:
        with tc.tile_pool(name="sing", bufs=1) as sing, \
             tc.tile_pool(name="fill", bufs=1, space="PSUM") as fillps, \
             tc.tile_pool(name="stps", bufs=2, space="PSUM") as stps, \
             tc.tile_pool(name="ytps", bufs=2, space="PSUM") as ytps, \
             tc.tile_pool(name="esb", bufs=4) as esb, \
             tc.tile_pool(name="nrm", bufs=6) as nrm, \
             tc.tile_pool(name="osb", bufs=4) as osb:
            xt_sb = sing.tile([P, KC, T], BF)
            wq_sb = sing.tile([P, KC, C], BF)
            wk_sb = sing.tile([P, KC, C], BF)
            wv_sb = sing.tile([P, KC, C], BF)
            wp_sb = sing.tile([P, 2, D], BF)
            tm_sb = sing.tile([P, P], BF)
            qt_sb = sing.tile([P, 2, T], BF)
            kt_sb = sing.tile([P, 2, T], BF)
            v_sb = sing.tile([P, 16, NHL, HD + 1], BF)
            yt_sb = sing.tile([P, 2, T], BF)
            bias_sb = sing.tile([P, 2, 2], F32)
            bv_row = sing.tile([1, C], F32)
            bvb_sb = sing.tile([P, C], F32)

            xt_r = xt_d.rearrange("(kc p) t -> kc p t", p=P)
            wq_r = wq_d.rearrange("(kc p) c -> p kc c", p=P)
            nc.sync.dma_start(wq_sb[:, :, 0:P], wq_r[:, :, 0:P])
            nc.sync.dma_start(wq_sb[:, :, P:C], wq_r[:, :, P:C])
            for kc in range(KC):
                nc.sync.dma_start(xt_sb[:, kc, :], xt_r[kc])
            nc.sync.dma_start(wk_sb[:], wk_d.rearrange("(kc p) c -> p kc c", p=P))
            nc.sync.dma_start(wv_sb[:], wv_d.rearrange("(kc p) c -> p kc c", p=P))
            nc.sync.dma_start(tm_sb[:], tm_d)
            nc.sync.dma_start(wp_sb[:], wp_d.rearrange("(cc p) o -> p cc o", p=P))
            nc.vector.memset(v_sb[:, :, :, HD:HD + 1], 1.0)
            nc.sync.dma_start(bias_sb[:], bqk_d.rearrange("p (cc r) -> p cc r", cc=2))
            nc.sync.dma_start(bv_row[:], bv_d)
            nc.gpsimd.partition_broadcast(bvb_sb[:], bv_row[:], channels=P)

            # ---- filler groups: QKV projections + output projection --------
            # Emitted into the attention loop to keep the PE fed while the
            # ACT-bound exp stream runs.  Each closure is one PSUM group.
            gidx = [0]

            Ident = mybir.ActivationFunctionType.Identity

            def qk_group(wsb, dst, cc, tb, ridx, use_act=False):
                def emit():
                    g = gidx[0]; gidx[0] += 1
                    ps = fillps.tile([P, 512], F32, tag=f"fl{g % 2}",
                                     name=f"qk_{g}")
                    for kc in range(KC):
                        nc.tensor.matmul(
                            ps[:],
                            wsb[:, kc, cc * P:(cc + 1) * P],
                            xt_sb[:, kc, tb * 512:(tb + 1) * 512],
                            start=(kc == 0), stop=(kc == KC - 1))
                    if use_act:
                        nc.scalar.activation(
                            dst[:, cc, tb * 512:(tb + 1) * 512], ps[:],
                            Ident, bias=bias_sb[:, cc, ridx:ridx + 1])
                    else:
                        nc.vector.tensor_scalar_add(
                            dst[:, cc, tb * 512:(tb + 1) * 512], ps[:],
                            bias_sb[:, cc, ridx:ridx + 1])
                return emit

            def v_group(ji):
                def emit():
                    g = gidx[0]; gidx[0] += 1
                    ps = fillps.tile([P, C], F32, tag=f"fl{g % 2}",
                                     name=f"v_{g}")
                    for kc in range(KC):
                        nc.tensor.matmul(
                            ps[:],
                            xt_sb[:, kc, ji * P:(ji + 1) * P],
                            wv_sb[:, kc, :],
                            start=(kc == 0), stop=(kc == KC - 1))
                    nc.vector.tensor_add(
                        v_sb[:, ji, :, 0:HD],
                        ps[:].rearrange("p (h x) -> p h x", h=NHL),
                        bvb_sb[:].rearrange("p (h x) -> p h x", h=NHL))
                return emit

            def proj_group(t2, ob, use_act=False):
                def emit():
                    g = gidx[0]; gidx[0] += 1
                    pp = fillps.tile([P, 512], F32, tag=f"fl{g % 2}",
                                     name=f"pr_{g}")
                    for cc in range(2):
                        nc.tensor.matmul(
                            pp[:],
                            yt_sb[:, cc, t2 * P:(t2 + 1) * P],
                            wp_sb[:, cc, ob * 512:(ob + 1) * 512],
                            start=(cc == 0), stop=(cc == 1))
                    ot = osb.tile([P, 512], BF, tag="ot", name=f"ot_{g}")
                    if use_act:
                        nc.scalar.copy(ot[:], pp[:])
                    else:
                        nc.vector.tensor_copy(ot[:], pp[:])
                    nc.sync.dma_start(
                        out_d[t2 * P:(t2 + 1) * P, ob * 512:(ob + 1) * 512],
                        ot[:])
                return emit

            def stage_groups(tb, prelude=False):
                # cc=0 (the first head pair) first so attention can start as
                # soon as its q/k copies land; prelude q-copies go to the
                # Scalar engine so they run in parallel with the k-copies
                # on the Vector engine.
                gs = [qk_group(wq_sb, qt_sb, 0, tb, 0, use_act=prelude),
                      qk_group(wk_sb, kt_sb, 0, tb, 1),
                      qk_group(wq_sb, qt_sb, 1, tb, 0, use_act=prelude),
                      qk_group(wk_sb, kt_sb, 1, tb, 1)]
                for ji in range(4 * tb, 4 * tb + 4):
                    gs.append(v_group(ji))
                return gs

            # prelude: everything attention(tb=0) needs
            for g in stage_groups(0, prelude=True):
                g()

            # ---- attention with interleaved fillers ------------------------
            for tb in range(4):
                tsl = slice(tb * 512, (tb + 1) * 512)
                fillers = []
                if tb < 3:
                    fillers += stage_groups(tb + 1)
                if tb >= 1:
                    for t2 in range(4 * (tb - 1), 4 * tb):
                        for ob in range(2):
                            fillers.append(proj_group(t2, ob))
                njc = 4 * tb + 4
                total_iters = 2 * njc
                it = 0
                done = 0
                for hp in (0, 2):
                    pair = (hp, hp + 1)
                    yps = {h: ytps.tile([P, 512], F32, tag="yt",
                                        name=f"yt_{tb}_{h}")
                           for h in pair}

                    def mm1_pair(ji, _tb=tb, _hp=hp, _pair=pair):
                        off = max(0, (ji - 4 * _tb) * P)
                        stp = stps.tile([P, 1024], F32, tag="st",
                                        name=f"st_{_tb}_{_hp}_{ji}")
                        for h in _pair:
                            bse = 64 * (h % 2)
                            cc = h // 2
                            nc.tensor.matmul(
                                stp[:, 512 * (h - _hp) + off:512 * (h - _hp) + 512],
                                kt_sb[bse:bse + 64, cc, ji * P:(ji + 1) * P],
                                qt_sb[bse:bse + 64, cc,
                                      _tb * 512 + off:(_tb + 1) * 512],
                                start=True, stop=True)
                        return stp, off

                    nxt = mm1_pair(0)
                    for ji in range(njc):
                        stp, off = nxt
                        if ji + 1 < njc:
                            nxt = mm1_pair(ji + 1)
                        et = esb.tile([P, 1024], BF, tag="et",
                                      name=f"et_{tb}_{hp}_{ji}")
                        stp3 = stp[:].rearrange("p (g c) -> p g c", g=2)
                        et3 = et[:].rearrange("p (g c) -> p g c", g=2)
                        nc.scalar.activation(et3[:, :, off:], stp3[:, :, off:],
                                             Exp, scale=0.125)
                        if ji >= 4 * tb:
                            nc.vector.tensor_mul(
                                et3[:, :, off:off + P],
                                et3[:, :, off:off + P],
                                tm_sb[:, None, :].to_broadcast([P, 2, P]))
                        for h in pair:
                            nc.tensor.matmul(
                                yps[h][:HD + 1, off:],
                                v_sb[:, ji, h, :],
                                et[:, 512 * (h - hp) + off:512 * (h - hp) + 512],
                                start=(ji == 0),
                                stop=(True if (tb == 3 and hp == 2)
                                      else ji == njc - 1),
                                skip_group_check=True)
                        if tb == 3 and hp == 2 and ji >= njc - 4:
                            # the j-chunks beyond ji no longer touch columns
                            # < 128*(ji-11), so that quarter of yT_aug is
                            # final: normalize it and chase with its output
                            # projection to keep the kernel tail short
                            qtr = ji - (njc - 4)
                            qs = slice(qtr * P, (qtr + 1) * P)
                            for h in pair:
                                bse = 64 * (h % 2)
                                rec = nrm.tile([1, P], F32, tag="recq",
                                               name=f"recq_{qtr}_{h}")
                                nc.vector.reciprocal(rec[:],
                                                     yps[h][HD:HD + 1, qs])
                                bc = nrm.tile([HD, P], F32, tag="bcq",
                                              name=f"bcq_{qtr}_{h}")
                                nc.gpsimd.partition_broadcast(bc[:], rec[:],
                                                              channels=HD)
                                nc.vector.tensor_mul(
                                    yt_sb[bse:bse + 64, 1,
                                          1536 + qtr * P:1536 + (qtr + 1) * P],
                                    yps[h][0:HD, qs], bc[:])
                            for ob in range(2):
                                proj_group(12 + qtr, ob, use_act=True)()
                        it += 1
                        while done * total_iters < len(fillers) * it:
                            fillers[done]()
                            done += 1
                    if tb == 3 and hp == 2:
                        continue  # fast tail path below
                    # normalization: get yT out of PSUM fast, then divide by
                    # the softmax denominator (row 64) broadcast across rows.
                    # The denominator row is DMA-reshaped to [128, 4] so the
                    # DVE reciprocal runs at free-size 4 instead of 512.
                    for h in pair:
                        bse = 64 * (h % 2)
                        cc = h // 2
                        ya = nrm.tile([HD + 1, 512], F32, tag="ya",
                                      name=f"ya_{tb}_{h}")
                        nc.vector.tensor_copy(ya[:], yps[h][:HD + 1, :])
                        rr = nrm.tile([P, 4], F32, tag="rr",
                                      name=f"rr_{tb}_{h}")
                        nc.sync.dma_start(rr[:], ya[HD:HD + 1, :])
                        rc = nrm.tile([P, 4], F32, tag="rc",
                                      name=f"rc_{tb}_{h}")
                        nc.vector.reciprocal(rc[:], rr[:])
                        rec = nrm.tile([1, 512], F32, tag="rec",
                                      name=f"rec_{tb}_{h}")
                        nc.sync.dma_start(rec[:], rc[:])
                        bc = nrm.tile([HD, 512], F32, tag="bc",
                                      name=f"bc_{tb}_{h}")
                        nc.gpsimd.partition_broadcast(bc[:], rec[:], channels=HD)
                        nc.vector.tensor_mul(yt_sb[bse:bse + 64, cc, tsl],
                                             ya[0:HD, :], bc[:])
                while done < len(fillers):
                    fillers[done]()
                    done += 1



    nc.compile()
    return nc


_NC = None


def _get_nc():
    global _NC
    if _NC is None:
        _NC = build_graph()
    return _NC


def make_in_maps(x, Wq, bq, Wk, bk, Wv, bv, Wp, bp):
    x = np.asarray(x, np.float32)
    tm = np.triu(np.ones((P, P), np.float32)).astype(bf16)  # keep where p <= f
    in_maps = []
    for core in range(N_CORES):
        b = core // 4
        hg = core % 4
        rs = slice(hg * C, (hg + 1) * C)
        xt = np.ascontiguousarray(x[b].T).astype(bf16)
        m = {"xt": xt, "tm": tm}
        for name, W in (("wq", Wq), ("wk", Wk), ("wv", Wv)):
            m[name] = np.ascontiguousarray(
                np.asarray(W, np.float32)[rs].T).astype(bf16)
        m["wp"] = np.ascontiguousarray(
            np.asarray(Wp, np.float32)[:, rs].T).astype(bf16)
        bqs = np.asarray(bq, np.float32)[rs].reshape(2, P)
        bks = np.asarray(bk, np.float32)[rs].reshape(2, P)
        m["bqk"] = np.stack([bqs[0], bks[0], bqs[1], bks[1]], axis=1)
        m["bv"] = np.asarray(bv, np.float32)[rs].reshape(1, C)
        in_maps.append(m)
    return in_maps


def kernel(x, Wq, bq, Wk, bk, Wv, bv, Wp, bp, _trace=False):
    nc = _get_nc()
    in_maps = make_in_maps(x, Wq, bq, Wk, bk, Wv, bv, Wp, bp)
    res = bass_utils.run_bass_kernel_spmd(
        nc, in_maps, core_ids=list(range(N_CORES)), trace=_trace)
    kernel.last_exec_time_ns = res.exec_time_ns
    bp = np.asarray(bp, np.float32)
    out = np.empty((B, T, D), np.float32)
    for b in range(B):
        acc = np.zeros((T, D), np.float32)
        for hg in range(4):
            acc += res.results[4 * b + hg]["out"].astype(np.float32)
        out[b] = acc + bp
    return out
